# revision 4
# baseline (speedup 1.0000x reference)
"""Trainium2 Bass kernel for DeepSets-style segment reduce (sum | mean | max).

Problem: x [1_000_000, 128] f32, batch [1_000_000] sorted int segment ids in
[0, 4096), output [4096, 384] = concat(seg_sum, seg_mean, seg_max).

Strategy (8 NeuronCores, no collectives needed):
  - Shard by SEGMENT ranges: core c owns segments [512c, 512(c+1)). Since batch
    is sorted, each core's rows are one contiguous slice of x.
  - Host packs each core's rows into a "binned" DRAM buffer: 4 windows of 128
    segments; inside a window each segment's rows are contiguous and padded
    with zero rows to a multiple of 16 (pads are zeros => sums exact).
  - Device (per window): dma_gather pulls each segment's rows into its own
    SBUF partition as 17 slots x 16 rows x 128 feat.  Per-partition =
    per-segment reductions are then pure free-axis ops:
      * max:  VectorE tensor_reduce per 16-row slot, invalid slots masked to
              -3e38 with a per-(partition,slot) mask, then folded over slots.
      * sum:  PE matmul with a stationary identity (fp32r) accumulates the 17
              slots into PSUM [128, 16*128]; VectorE folds the last 16 rows.
      * mean: sum * (1/count) with a per-partition scalar.
  - Host finishes: segments with >272 rows (p~4% for the spec's distribution)
    are computed exactly on host and overwritten; empty segments clamp to 0 on
    device.
"""

import time
from contextlib import ExitStack

import numpy as np

import concourse.bass as bass
import concourse.tile as tile
from concourse import bacc, mybir
from concourse.bass_utils import run_bass_kernel_spmd
from concourse.masks import make_identity

# ---- problem constants (hardcoded per spec) ----
N_ROWS = 1_000_000
H = 128
B = 4096
NCORES = 8
P = 128

SEGS_PER_CORE = B // NCORES          # 512
NW = 4                               # windows (of 128 segments) per core
WROWS = 36864                        # buffer rows reserved per window (even)
E_A = 17                             # 16-row slots per segment on device
CAP = 16 * E_A                       # 272 device-covered rows per segment
CHUNKS = (4, 4, 4, 5)                # slots per gather chunk (sum = E_A)
ZROW = 36700                         # zero row inside each window slot (even)
BIGF = 3.0e38

F32 = mybir.dt.float32
F32R = mybir.dt.float32r
I16 = mybir.dt.int16


def build_module(reps: int = 1):
    """Build the SPMD per-core Bass module. reps>1 wraps the body in a loop
    (used only for timing)."""
    nc = bacc.Bacc(
        "TRN2", target_bir_lowering=False, debug=False, enable_asserts=True,
        num_devices=NCORES,
    )
    buf = nc.dram_tensor("buf", [NW * WROWS, H], F32, kind="ExternalInput").ap()
    idx = nc.dram_tensor("idx", [NW, P, 8 * E_A], I16, kind="ExternalInput").ap()
    pf = nc.dram_tensor("pf", [NW, P, 20], F32, kind="ExternalInput").ap()
    out = nc.dram_tensor("out", [NW * P, 3 * H], F32, kind="ExternalOutput").ap()

    with tile.TileContext(nc) as tc, ExitStack() as ctx:
        cpool = ctx.enter_context(tc.tile_pool(name="consts", bufs=1))
        ipool = ctx.enter_context(tc.tile_pool(name="idxp", bufs=2))
        ppool = ctx.enter_context(tc.tile_pool(name="pfp", bufs=2))
        gpool = ctx.enter_context(tc.tile_pool(name="gath", bufs=3))
        smpool = ctx.enter_context(tc.tile_pool(name="slotmax", bufs=2))
        mkpool = ctx.enter_context(tc.tile_pool(name="masked", bufs=2))
        wpool = ctx.enter_context(tc.tile_pool(name="small", bufs=2))
        opool = ctx.enter_context(tc.tile_pool(name="outt", bufs=2))
        pspool = ctx.enter_context(
            tc.tile_pool(name="psum", bufs=2, space="PSUM")
        )

        ident = cpool.tile([P, P], F32)
        make_identity(nc, ident[:])
        identr_t = cpool.tile([P, P], F32R)
        nc.vector.tensor_copy(out=identr_t[:], in_=ident[:])
        identr = identr_t[:]

        def window_body(w: int):
            idxt = ipool.tile([P, 8 * E_A], I16)
            nc.sync.dma_start(out=idxt[:], in_=idx[w])
            pt = ppool.tile([P, 20], F32)
            nc.sync.dma_start(out=pt[:], in_=pf[w])

            smt = smpool.tile([P, E_A, H], F32)
            pst = pspool.tile([P, 16 * H], F32)

            src = bass.AP(
                buf.tensor, w * WROWS * H, [[256, (WROWS - 16) // 2], [1, 2048]]
            ).bitcast(F32R)

            j0 = 0
            for ec in CHUNKS:
                gt = gpool.tile([P, max(CHUNKS), 2048], F32R)
                nc.gpsimd.dma_gather(
                    out_ap=gt[:, 0:ec, :],
                    in_ap=src,
                    idxs_ap=idxt[:, 8 * j0:8 * (j0 + ec)],
                    num_idxs=P * ec,
                    num_idxs_reg=P * ec,
                    elem_size=2048,
                    elem_step=256,
                )
                # per-slot max over the 16 rows: view [p, slot, feat, row]
                gv = gt[:, 0:ec, :].bitcast(F32).rearrange(
                    "p s (r f) -> p s f r", r=16, f=H
                )
                nc.vector.tensor_reduce(
                    out=smt[:, j0:j0 + ec, :], in_=gv,
                    axis=mybir.AxisListType.X, op=mybir.AluOpType.max,
                )
                # slot-sum on PE: psum[p, r*128+f] += slot (identity matmul)
                for s in range(ec):
                    jg = j0 + s
                    for q in range(4):
                        nc.tensor.matmul(
                            out=pst[:, 512 * q:512 * (q + 1)],
                            lhsT=identr,
                            rhs=gt[:, s, 512 * q:512 * (q + 1)],
                            start=(jg == 0),
                            stop=(jg == E_A - 1),
                        )
                j0 += ec

            ptap = pt[:]
            maskb = bass.AP(ptap.tensor, ptap.offset, [[20, P], [1, E_A], [0, H]])
            mk = mkpool.tile([P, E_A, H], F32)
            nc.vector.tensor_tensor(
                out=mk[:], in0=smt[:], in1=maskb, op=mybir.AluOpType.min
            )
            wm = wpool.tile([P, H], F32)
            nc.vector.tensor_reduce(
                out=wm[:], in_=mk[:].rearrange("p s f -> p f s"),
                axis=mybir.AxisListType.X, op=mybir.AluOpType.max,
            )

            ot = opool.tile([P, 3 * H], F32)
            tc1 = wpool.tile([P, H], F32)
            nc.vector.tensor_scalar_min(out=tc1[:], in0=wm[:], scalar1=pt[:, 17:18])
            nc.vector.tensor_scalar_max(
                out=ot[:, 2 * H:3 * H], in0=tc1[:], scalar1=pt[:, 18:19]
            )
            # fold the 16 rows of the PE slot-sum: view [p, feat, row]
            nc.vector.tensor_reduce(
                out=ot[:, 0:H], in_=pst[:].rearrange("p (r f) -> p f r", r=16, f=H),
                axis=mybir.AxisListType.X, op=mybir.AluOpType.add,
            )
            nc.vector.tensor_scalar_mul(
                out=ot[:, H:2 * H], in0=ot[:, 0:H], scalar1=pt[:, 19:20]
            )
            nc.sync.dma_start(out=out[P * w:P * (w + 1), :], in_=ot[:])

        if reps == 1:
            for w in range(NW):
                window_body(w)
        else:
            with tc.For_i(0, reps, 1):
                for w in range(NW):
                    window_body(w)

    nc.compile()
    return nc


# ---------------- host side ----------------

def _np_reference(x, batch):
    """Pure-numpy exact fallback (used only for assumption violations)."""
    counts = np.bincount(batch, minlength=B)
    starts = np.concatenate([[0], np.cumsum(counts)[:-1]]).astype(np.int64)
    sums = np.zeros((B, H), np.float32)
    maxs = np.zeros((B, H), np.float32)
    nz = counts > 0
    if nz.any():
        bidx = starts[nz]
        sums[nz] = np.add.reduceat(x, bidx, axis=0)[: nz.sum()]
        maxs[nz] = np.maximum.reduceat(x, bidx, axis=0)[: nz.sum()]
    means = sums / np.maximum(counts, 1)[:, None]
    return np.concatenate([sums, means, maxs], axis=1).astype(np.float32)


def host_prep(x, batch):
    x = np.ascontiguousarray(np.asarray(x, dtype=np.float32))
    b = np.asarray(batch).astype(np.int64).ravel()
    counts = np.bincount(b, minlength=B).astype(np.int64)
    starts = (np.cumsum(counts) - counts).astype(np.int64)

    used = np.minimum(counts, CAP)
    cpad = np.minimum(((counts + 15) // 16) * 16, CAP)
    nslots = cpad // 16
    big = np.where(counts > CAP)[0]

    cpad_w = cpad.reshape(NCORES, NW, P)
    off_w = (np.cumsum(cpad_w, axis=2) - cpad_w).astype(np.int64)  # exclusive

    bufs = np.zeros((NCORES, NW * WROWS, H), np.float32)
    ridx = np.arange(len(b), dtype=np.int64) - starts[b]
    keep = ridx < used[b]
    g = b[keep]
    rk = ridx[keep]
    core = g // SEGS_PER_CORE
    w = (g % SEGS_PER_CORE) // P
    p = g % P
    dstrow = w * WROWS + off_w[core, w, p] + rk
    bufs.reshape(NCORES * NW * WROWS, H)[core * (NW * WROWS) + dstrow] = x[keep]

    slots = np.arange(E_A, dtype=np.int64)
    idxv = (off_w // 2)[..., None] + 8 * slots  # [8, NW, P, E_A]
    validm = slots[None, None, None, :] < nslots.reshape(NCORES, NW, P)[..., None]
    idxv = np.where(validm, idxv, ZROW // 2).astype(np.int16)
    flat = idxv.transpose(0, 1, 3, 2).reshape(NCORES, NW, E_A * P)  # i = j*128+p
    wrapped = flat.reshape(NCORES, NW, (E_A * P) // 16, 16).transpose(0, 1, 3, 2)
    idx_in = np.ascontiguousarray(np.tile(wrapped, (1, 1, 8, 1)))  # [8, NW, 128, 136]

    maskv = np.where(validm, BIGF, -BIGF).astype(np.float32)
    nonempty = (counts > 0).reshape(NCORES, NW, P)
    hi = np.where(nonempty, BIGF, 0.0).astype(np.float32)
    lo = np.where(nonempty, -BIGF, 0.0).astype(np.float32)
    inv = (1.0 / np.maximum(counts, 1)).astype(np.float32).reshape(NCORES, NW, P)
    pfv = np.concatenate(
        [maskv, hi[..., None], lo[..., None], inv[..., None]], axis=3
    )  # [8, NW, 128, 20]

    in_maps = [
        {"buf": bufs[c], "idx": idx_in[c], "pf": np.ascontiguousarray(pfv[c])}
        for c in range(NCORES)
    ]
    return x, b, counts, starts, big, in_maps


def assemble(results, x, counts, starts, big):
    out = np.concatenate([r["out"] for r in results], axis=0)
    # exact host fix-up for segments the device only partially covered
    for s in big:
        xs = x[starts[s]:starts[s] + counts[s]]
        sm = xs.sum(axis=0, dtype=np.float32)
        out[s, 0:H] = sm
        out[s, H:2 * H] = sm / np.float32(counts[s])
        out[s, 2 * H:3 * H] = xs.max(axis=0)
    return out


_NC_CACHE = {}


def kernel(x, batch, batch_size):
    x = np.asarray(x)
    b = np.asarray(batch).ravel()
    if (
        int(batch_size) != B
        or x.shape != (N_ROWS, H)
        or b.shape[0] != N_ROWS
        or np.any(b[1:] < b[:-1])
    ):
        return _np_reference(
            np.asarray(x, dtype=np.float32), b.astype(np.int64)
        )

    xf, b64, counts, starts, big, in_maps = host_prep(x, b)

    if "nc" not in _NC_CACHE:
        _NC_CACHE["nc"] = build_module(reps=1)
    nc = _NC_CACHE["nc"]

    res = run_bass_kernel_spmd(nc, in_maps, list(range(NCORES)))
    return assemble(res.results, xf, counts, starts, big)


if __name__ == "__main__":
    t0 = time.time()
    rng = np.random.default_rng(0)
    x = rng.standard_normal((N_ROWS, H), dtype=np.float32)
    batch = np.sort(rng.integers(0, B, N_ROWS).astype(np.int32))
    print("gen", time.time() - t0)
    t0 = time.time()
    out = kernel(x=x, batch=batch, batch_size=B)
    print("kernel", time.time() - t0, out.shape, out.dtype)
